# revision 17
# baseline (speedup 1.0000x reference)
"""Trainium2 Bass kernel for CrossStockAttention (sparse similarity-top-k attention).

Sharding: 8 cores = 2 batches x 4 query-row blocks of 512. Each core receives
the full key set of its batch plus its query slice and computes everything
on-chip. No collectives.

v2 restructure vs baseline:
- top-k selection is invariant to positive per-row scaling, so the cosine-sim
  matrix is replaced by the raw gram matrix G = xq @ x^T (fp16 on PE) scaled
  only by per-key 1/||x_k|| (host-precomputed). Kills both normalize rounds.
- invalid keys pushed out of the ranking by a rank-1 (-2000 per invalid key)
  PE accumulation instead of a DVE add pass.
- top-40 threshold via 2-level max8: per-64 block max8 -> 256 candidates ->
  5 rounds max8/match_replace (exact unless one 64-block holds >8 of a row's
  top-40; ~1e-4 probability per row).
- binary allowed-mask (fp16, transposed [keys, q]); applied either on DVE as
  psum += 256*allowed before exp(scale*psum - 256*scale) (blocked entries
  underflow to exactly 0), or as a GpSimd post-exp multiply.
- weights shipped fp16 from host; per-key norms host-precomputed.
"""

import numpy as np

B, N, D, H = 2, 2048, 256, 8
DH = D // H            # 32
TOPK = 40
P = 128
NCORES = 8
QS = 512               # query rows per core
NT = N // P            # 16 key row-tiles
QT = QS // P           # 4 query row-tiles
DC = D // P            # 2 contraction chunks of 128
LN_EPS = 1e-5
SCALE = 1.0 / DH ** 0.5
KB_NEG = -2000.0       # rank-1 sim bias for invalid keys
MB = 256.0             # mask bias: psum += MB * allowed, exp bias -MB*SCALE
TPAD = 50.0            # threshold for padding queries = -TPAD
MR_MIN = -2e9
BLK = 128              # L1 top-k block size
NBLK = N // BLK        # 16
NCAND = NBLK * 8       # 128

_CACHE = {}


def _emit(nc, tc, ctx):
    import concourse.bass as bass
    import concourse.mybir as mybir
    from concourse.masks import make_identity

    f32 = mybir.dt.float32
    f16 = mybir.dt.float16
    AF = mybir.ActivationFunctionType
    OP = mybir.AluOpType

    xq_d = nc.dram_tensor("xq", [QS, D], f32, kind="ExternalInput")
    w_d = {}
    for nm in ("wq", "wk", "wv", "wo"):
        w_d[nm] = nc.dram_tensor(nm, [D, D], f16, kind="ExternalInput")
    b_d = {}
    for nm in ("bq", "bk", "bv", "bo", "g", "bt"):
        b_d[nm] = nc.dram_tensor(nm, [D], f32, kind="ExternalInput")
    xt16_d = nc.dram_tensor("xt16", [D, N], f16, kind="ExternalInput")
    xqt16_d = nc.dram_tensor("xqt16", [D, QS], f16, kind="ExternalInput")
    kb_d = nc.dram_tensor("kb16", [1, N], f16, kind="ExternalInput")
    rn_d = nc.dram_tensor("rn16", [1, N], f16, kind="ExternalInput")
    qv_d = nc.dram_tensor("qv", [P, QT], f32, kind="ExternalInput")
    out_d = nc.dram_tensor("out", [QS, D], f32, kind="ExternalOutput")

    def bcast_ap(handle, n_part):
        ap = handle.ap()
        return bass.AP(tensor=ap.tensor, offset=ap.offset,
                       ap=[[0, n_part]] + [list(p) for p in ap.ap])

    consts = ctx.enter_context(tc.tile_pool(name="consts", bufs=1))
    big = ctx.enter_context(tc.tile_pool(name="big", bufs=1))
    share = ctx.enter_context(tc.tile_pool(name="share", bufs=3))
    simp = ctx.enter_context(tc.tile_pool(name="simp", bufs=2))
    scrp = ctx.enter_context(tc.tile_pool(name="scrp", bufs=2))
    small = ctx.enter_context(tc.tile_pool(name="small", bufs=4))
    psA = ctx.enter_context(tc.tile_pool(name="psA", bufs=2, space="PSUM"))
    psT = ctx.enter_context(tc.tile_pool(name="psT", bufs=2, space="PSUM"))
    psO = ctx.enter_context(tc.tile_pool(name="psO", bufs=2, space="PSUM"))

    # ---------------- constants / weights ----------------
    ident = consts.tile([P, P], f32, tag="ident")
    make_identity(nc, ident)
    ident16 = consts.tile([P, P], f16, tag="ident16")
    nc.vector.tensor_copy(ident16, ident)
    ones_row = consts.tile([1, P], f16, tag="ones_row")
    nc.vector.memset(ones_row, 1.0)
    ebias = consts.tile([P, 1], f32, tag="ebias")
    nc.vector.memset(ebias, float(-MB * SCALE))
    identPos = consts.tile([P, P], f16, tag="identPos")
    nc.vector.tensor_scalar(identPos, ident16, float(MB), None, op0=OP.mult)

    w16 = {}
    for nm in ("wq", "wk", "wv", "wo"):
        w16[nm] = consts.tile([P, DC, D], f16, tag=f"w16_{nm}", name=f"w16_{nm}")
        for dc in range(DC):
            nc.sync.dma_start(out=w16[nm][:, dc, :], in_=w_d[nm][dc * P:(dc + 1) * P, :])
    bT = {}
    for nm in ("bq", "bk", "bo"):
        bT[nm] = consts.tile([P, DC], f32, tag=f"bT_{nm}", name=f"bT_{nm}")
        b2 = b_d[nm].ap().rearrange("(ec p) -> ec p", ec=DC)
        for ec in range(DC):
            nc.sync.dma_start(out=bT[nm][:, ec:ec + 1], in_=b2[ec:ec + 1, :])
    bv_rep = consts.tile([P, D], f32, tag="bv_rep")
    nc.gpsimd.dma_start(out=bv_rep, in_=bcast_ap(b_d["bv"], P))
    g_rep = consts.tile([P, D], f32, tag="g_rep")
    nc.gpsimd.dma_start(out=g_rep, in_=bcast_ap(b_d["g"], P))
    bt_rep = consts.tile([P, D], f32, tag="bt_rep")
    nc.gpsimd.dma_start(out=bt_rep, in_=bcast_ap(b_d["bt"], P))
    qv_sb = consts.tile([P, QT], f32, tag="qv_sb")
    nc.sync.dma_start(out=qv_sb, in_=qv_d[:, :])
    kbv = consts.tile([1, N], f16, tag="kbv")
    nc.sync.dma_start(out=kbv, in_=kb_d[:, :])
    rn16 = consts.tile([1, N], f16, tag="rn16")
    nc.sync.dma_start(out=rn16, in_=rn_d[:, :])

    xT = big.tile([P, DC, N], f16, tag="xT")
    nc.sync.dma_start(out=xT, in_=xt16_d.ap().rearrange("(dc p) j -> p dc j", p=P))
    xqT = big.tile([P, DC, QS], f16, tag="xqT")
    nc.sync.dma_start(out=xqT, in_=xqt16_d.ap().rearrange("(dc p) j -> p dc j", p=P))
    xq_rows = big.tile([P, QT, D], f32, tag="xq_rows")
    nc.sync.dma_start(out=xq_rows, in_=xq_d.ap().rearrange("(t p) d -> p t d", p=P))

    # per-key 1/||x_k|| broadcast to all partitions via PE rank-1
    rnk_rep = big.tile([P, N], f32, tag="rnk_rep")
    for jg in range(2):
        pr = psA.tile([P, 2, 512], f32, tag="psA", name=f"pr_{jg}")
        for k in range(2):
            ch = jg * 2 + k
            nc.tensor.matmul(pr[:, k, :], lhsT=ones_row,
                             rhs=rn16[:, ch * 512:(ch + 1) * 512],
                             start=True, stop=True)
        nc.scalar.copy(rnk_rep[:, jg * 1024:(jg + 1) * 1024], pr)

    # ---------------- sim (scaled gram), top-k threshold, mask ----------------
    maskA = big.tile([P, NT, QS], f16, tag="maskA")   # 1.0 = allowed, [keys, q]
    for t in range(QT):
        sim_m = simp.tile([P, N], f32, tag="simm", name=f"simm_{t}")
        for jg in range(2):
            ps = psA.tile([P, 2, 512], f32, tag="psA", name=f"psim_{t}_{jg}")
            for k in range(2):
                jc = jg * 2 + k
                for dc in range(DC):
                    nc.tensor.matmul(
                        ps[:, k, :],
                        lhsT=xqT[:, dc, t * P:(t + 1) * P],
                        rhs=xT[:, dc, jc * 512:(jc + 1) * 512],
                        start=dc == 0, stop=False)
                nc.tensor.matmul(ps[:, k, :], lhsT=ones_row,
                                 rhs=kbv[:, jc * 512:(jc + 1) * 512],
                                 start=False, stop=True)
            nc.vector.tensor_mul(sim_m[:, jg * 1024:(jg + 1) * 1024], ps,
                                 rnk_rep[:, jg * 1024:(jg + 1) * 1024])
        # L1: per-64-block max8 -> 256 candidates
        cand = small.tile([P, NBLK, 8], f32, tag="cand", name=f"cand_{t}")
        for blk in range(NBLK):
            nc.vector.max(cand[:, blk, :], sim_m[:, blk * BLK:(blk + 1) * BLK])
        # L2: exact top-40 of candidates
        scr = small.tile([P, NCAND], f32, tag="scr", name=f"scr_{t}")
        mx = None
        for it in range(5):
            mx = small.tile([P, 8], f32, tag="mx8", name=f"mx_{t}_{it}")
            src = cand if it == 0 else scr
            nc.vector.max(mx, src)
            if it < 4:
                nc.vector.match_replace(scr, mx, src, MR_MIN)
        # T' = T40*qv + (qv-1)*TPAD: exact T40 for valid rows, -TPAD for padding
        tS = small.tile([P, 1], f32, tag="tS", name=f"tS_{t}")
        tP = small.tile([P, 1], f32, tag="tP", name=f"tP_{t}")
        nc.vector.tensor_scalar(tP, qv_sb[:, t:t + 1], 1.0, float(TPAD),
                                op0=OP.subtract, op1=OP.mult)
        nc.vector.tensor_mul(tS, mx[:, 7:8], qv_sb[:, t:t + 1])
        nc.vector.tensor_add(tS, tS, tP)
        # mrow = (sim >= T)   (fp16 row-major), then transpose to [keys, q]
        mrow = scrp.tile([P, N], f16, tag="mrow", name=f"mrow_{t}")
        nc.vector.tensor_scalar(mrow, sim_m, tS, None, op0=OP.is_ge)
        for g in range(NT // 4):
            pt = psT.tile([P, 4, P], f16, tag="psT", name=f"ptm_{t}_{g}")
            for kk in range(4):
                jt = g * 4 + kk
                nc.tensor.transpose(pt[:, kk, :], mrow[:, jt * P:(jt + 1) * P], ident16)
            nc.scalar.copy(maskA[:, g * 4:(g + 1) * 4, t * P:(t + 1) * P], pt)

    # ---------------- projections ----------------
    kT = big.tile([P, DC, N], f16, tag="kT")
    qT = big.tile([P, DC, QS], f16, tag="qT")
    v_aug = big.tile([P, NT, H, DH + 1], f16, tag="v_aug")
    nc.vector.memset(v_aug[:, :, :, DH:DH + 1], 1.0)
    bv_hd = bv_rep.rearrange("p (h d) -> p h d", h=H)

    for ec in range(DC):
        for jg in range(N // 1024):
            pk = psA.tile([P, 2, 512], f32, tag="psA", name=f"pk_{ec}_{jg}")
            for k in range(2):
                jc = jg * 2 + k
                for dc in range(DC):
                    nc.tensor.matmul(
                        pk[:, k, :],
                        lhsT=w16["wk"][:, dc, ec * P:(ec + 1) * P],
                        rhs=xT[:, dc, jc * 512:(jc + 1) * 512],
                        start=dc == 0, stop=dc == DC - 1)
            nc.scalar.activation(kT[:, ec, jg * 1024:(jg + 1) * 1024], pk,
                                 AF.Identity, bias=bT["bk"][:, ec:ec + 1])

    pq = psA.tile([P, 2, 512], f32, tag="psA", name="pq")
    for ec in range(DC):
        for dc in range(DC):
            nc.tensor.matmul(
                pq[:, ec, :],
                lhsT=w16["wq"][:, dc, ec * P:(ec + 1) * P],
                rhs=xqT[:, dc, :],
                start=dc == 0, stop=dc == DC - 1)
    for ec in range(DC):
        nc.scalar.activation(qT[:, ec, :], pq[:, ec, :], AF.Identity,
                             bias=bT["bq"][:, ec:ec + 1])

    for jg in range(NT // 2):
        pv = psA.tile([P, 2, 512], f32, tag="psA", name=f"pv_{jg}")
        for k in range(2):
            jt = jg * 2 + k
            for dc in range(DC):
                nc.tensor.matmul(
                    pv[:, k, 0:D],
                    lhsT=xT[:, dc, jt * P:(jt + 1) * P],
                    rhs=w16["wv"][:, dc, :],
                    start=dc == 0, stop=dc == DC - 1)
        for k in range(2):
            jt = jg * 2 + k
            nc.vector.tensor_add(
                v_aug[:, jt, :, 0:DH],
                pv[:, k, 0:D].rearrange("p (h d) -> p h d", h=H),
                bv_hd)

    # ---------------- attention per head ----------------
    # emission order: first NGP heads use post-exp GpSimd masking (their exp
    # doesn't wait on the mask), the rest use a DVE psum-add before exp.
    outT = big.tile([P, DC, QS], f32, tag="outT")
    sumsA = big.tile([P, QS], f32, tag="sumsA")   # heads 0-3 at partitions 32h
    sumsB = big.tile([P, QS], f32, tag="sumsB")   # heads 4-7
    for h in range(H):
        ec, p0 = h // 4, DH * (h % 4)
        pe_mask = h % 2 == 0   # even heads: PE psum mask-mm; odd: DVE post-exp mul
        expT = share.tile([P, NT, QS], f16, tag="share", name=f"expT_{h}")
        for g in range(NT // 2):
            ps = psA.tile([P, 2, 512], f32, tag="psA", name=f"pst_{h}_{g}")
            if pe_mask:
                for k in range(2):
                    jt = g * 2 + k
                    nc.tensor.matmul(ps[:, k, :], lhsT=identPos, rhs=maskA[:, jt, :],
                                     start=True, stop=False)
            for k in range(2):
                jt = g * 2 + k
                nc.tensor.matmul(
                    ps[:, k, :],
                    lhsT=kT[p0:p0 + DH, ec, jt * P:(jt + 1) * P],
                    rhs=qT[p0:p0 + DH, ec, :],
                    start=not pe_mask, stop=True, tile_position=(p0, 0))
            if pe_mask:
                nc.scalar.activation(expT[:, g * 2:(g + 1) * 2, :], ps, AF.Exp,
                                     scale=float(SCALE), bias=ebias[:, 0:1])
            else:
                nc.scalar.activation(expT[:, g * 2:(g + 1) * 2, :], ps, AF.Exp,
                                     scale=float(SCALE))
                nc.vector.tensor_mul(expT[:, g * 2:(g + 1) * 2, :],
                                     expT[:, g * 2:(g + 1) * 2, :],
                                     maskA[:, g * 2:(g + 1) * 2, :])
        po = psO.tile([DH + 1, QS], f32, tag="psO", name=f"po_{h}")
        for jt in range(NT):
            nc.tensor.matmul(
                po,
                lhsT=v_aug[:, jt, h, :],
                rhs=expT[:, jt, :],
                start=jt == 0, stop=jt == NT - 1)
        nc.vector.tensor_copy(outT[p0:p0 + DH, ec, :], po[0:DH, :])
        nc.vector.tensor_copy((sumsA if h < 4 else sumsB)[p0:p0 + 1, :], po[DH:DH + 1, :])

    # transpose sums to row-major [i, h], divide, transpose back
    sums_rows = big.tile([P, QT, H], f32, tag="sums_rows")
    for it in range(QT):
        pt_s = psT.tile([P, 4, P], f32, tag="psT", name=f"pt_sums_{it}")
        nc.tensor.transpose(pt_s[:, 0, :], sumsA[:, it * P:(it + 1) * P], ident)
        nc.tensor.transpose(pt_s[:, 1, :], sumsB[:, it * P:(it + 1) * P], ident)
        for half in range(2):
            base = pt_s[:, half, :]
            src = bass.AP(tensor=base.tensor, offset=base.offset,
                          ap=[list(base.ap[0]), [DH, 4]])
            nc.scalar.copy(sums_rows[:, it, half * 4:half * 4 + 4], src)
    recip_rows = big.tile([P, QT, H], f32, tag="recip_rows")
    nc.vector.reciprocal(recip_rows, sums_rows)

    out_rows = big.tile([P, QT, D], f32, tag="out_rows")
    for ec in range(DC):
        pt = psT.tile([P, 4, P], f32, tag="psT", name=f"pto_{ec}")
        for it in range(QT):
            nc.tensor.transpose(pt[:, it, :], outT[:, ec, it * P:(it + 1) * P], ident)
        nc.scalar.copy(out_rows[:, 0:QT, ec * P:(ec + 1) * P], pt)
    for t in range(QT):
        rb = recip_rows[:, t, :]
        rb_b = bass.AP(tensor=rb.tensor, offset=rb.offset,
                       ap=[list(rb.ap[0])] + [list(rb.ap[-1]), [0, DH]])
        nc.vector.tensor_mul(out_rows[:, t, :].rearrange("p (h d) -> p h d", h=H),
                             out_rows[:, t, :].rearrange("p (h d) -> p h d", h=H),
                             rb_b)
    outT2 = big.tile([P, DC, QS], f16, tag="outT2")
    for ec in range(DC):
        pt = psT.tile([P, 4, P], f32, tag="psT", name=f"ptb_{ec}")
        for it in range(QT):
            nc.tensor.transpose(pt[:, it, :], out_rows[:, it, ec * P:(ec + 1) * P], ident)
        nc.scalar.copy(outT2[:, ec, :], pt)

    # ---------------- output projection, residual, LN ----------------
    finalT = big.tile([P, DC, QS], f32, tag="finalT")
    pf = psA.tile([P, 2, 512], f32, tag="psA", name="pf")
    for ec in range(DC):
        for dc in range(DC):
            nc.tensor.matmul(
                pf[:, ec, :],
                lhsT=w16["wo"][:, dc, ec * P:(ec + 1) * P],
                rhs=outT2[:, dc, :],
                start=dc == 0, stop=dc == DC - 1)
    for ec in range(DC):
        nc.vector.tensor_scalar_add(finalT[:, ec, :], pf[:, ec, :], bT["bo"][:, ec:ec + 1])

    fin = big.tile([P, QT, D], f32, tag="fin")
    for ec in range(DC):
        pt = psT.tile([P, 4, P], f32, tag="psT", name=f"ptf_{ec}")
        for it in range(QT):
            nc.tensor.transpose(pt[:, it, :], finalT[:, ec, it * P:(it + 1) * P], ident)
        nc.scalar.copy(fin[:, 0:QT, ec * P:(ec + 1) * P], pt)
    nc.vector.tensor_add(fin, fin, xq_rows)

    st6 = small.tile([P, QT, 6], f32, tag="st6")
    mv = small.tile([P, QT, 2], f32, tag="mv")
    for t in range(QT):
        nc.vector.bn_stats(st6[:, t, :], fin[:, t, :])
        nc.vector.bn_aggr(mv[:, t, :], st6[:, t, :])
    rstd = small.tile([P, QT, 1], f32, tag="rstd")
    nc.vector.tensor_scalar(rstd, mv[:, :, 1:2], float(LN_EPS), None, op0=OP.add)
    nc.scalar.activation(rstd, rstd, AF.Sqrt)
    nc.vector.reciprocal(rstd, rstd)
    for t in range(QT):
        nc.vector.tensor_scalar(fin[:, t, :], fin[:, t, :], mv[:, t, 0:1], rstd[:, t, 0:1],
                                op0=OP.subtract, op1=OP.mult)
        nc.vector.tensor_mul(fin[:, t, :], fin[:, t, :], g_rep)
        nc.vector.tensor_add(fin[:, t, :], fin[:, t, :], bt_rep)
        nc.sync.dma_start(out=out_d[t * P:(t + 1) * P, :], in_=fin[:, t, :])


def build_nc():
    from contextlib import ExitStack
    import concourse.bacc as bacc
    from concourse.tile import TileContext

    nc = bacc.Bacc("TRN2", target_bir_lowering=False, debug=False, num_devices=NCORES)
    with TileContext(nc) as tc:
        with ExitStack() as ctx:
            _emit(nc, tc, ctx)
    nc.compile()
    return nc


def _in_maps(inputs):
    x = np.asarray(inputs["stock_features"], dtype=np.float32)
    valid = np.asarray(inputs["stock_valid_mask"]).astype(bool)
    kb16 = np.where(valid, 0.0, KB_NEG).astype(np.float16)        # [B, N]
    rn16 = (1.0 / np.maximum(np.linalg.norm(x, axis=-1), 1e-12)).astype(np.float16)
    shared = {
        "wq": np.ascontiguousarray(inputs["w_q"], np.float16),
        "wk": np.ascontiguousarray(inputs["w_k"], np.float16),
        "wv": np.ascontiguousarray(inputs["w_v"], np.float16),
        "wo": np.ascontiguousarray(inputs["w_o"], np.float16),
        "bq": np.ascontiguousarray(inputs["b_q"], np.float32),
        "bk": np.ascontiguousarray(inputs["b_k"], np.float32),
        "bv": np.ascontiguousarray(inputs["b_v"], np.float32),
        "bo": np.ascontiguousarray(inputs["b_o"], np.float32),
        "g": np.ascontiguousarray(inputs["ln_g"], np.float32),
        "bt": np.ascontiguousarray(inputs["ln_b"], np.float32),
    }
    maps = []
    for c in range(NCORES):
        b, qi = divmod(c, 4)
        q0 = qi * QS
        qv = valid[b, q0:q0 + QS].astype(np.float32).reshape(QT, P).T.copy()
        m = dict(shared)
        m["xq"] = np.ascontiguousarray(x[b, q0:q0 + QS])
        xt16 = np.ascontiguousarray(x[b].T.astype(np.float16))
        m["xt16"] = xt16
        m["xqt16"] = np.ascontiguousarray(xt16[:, q0:q0 + QS])
        m["kb16"] = np.ascontiguousarray(kb16[b].reshape(1, N))
        m["rn16"] = np.ascontiguousarray(rn16[b].reshape(1, N))
        m["qv"] = qv
        maps.append(m)
    return maps


def kernel(**inputs):
    from concourse.bass_utils import run_bass_kernel_spmd

    if "nc" not in _CACHE:
        _CACHE["nc"] = build_nc()
    nc = _CACHE["nc"]
    res = run_bass_kernel_spmd(nc, _in_maps(inputs), list(range(NCORES)))
    out = np.empty((B, N, D), np.float32)
    for c in range(NCORES):
        b, qi = divmod(c, 4)
        out[b, qi * QS:(qi + 1) * QS] = res.results[c]["out"]
    return out
